# revision 3
# baseline (speedup 1.0000x reference)
"""Causal single-head attention (B=8, T=2048, D=1024, HS=64) on 8 TRN2 cores.

v5 = v2's proven single-dense-PE-stream interleave + dual-lane S matmuls.

Key facts driving the design (HW-measured):
  - One [128,1024] Scalar EXP = ~1.1us; total exp volume ~= S volume.
    The exp train (~20us) must run gapless UNDER overlapping PE work.
  - T0/T8 row-tiled K=64 matmuls run CONCURRENTLY (pair cadence ~260ns)
    and mix freely with 128-mode matmuls -> S^T tasks go dual-lane
    (2 per wall-slot).  PV stays SINGLE-lane K=128: that keeps ot to 2
    PSUM banks so proj1 can accumulate during the attention windows
    (PSUM budget: sp 2x2 + {pqk|pv|ot} rotation 2x2 = 8 banks).
  - Causal trims: S/exp/PV skip the fully-masked 128-col blocks of
    diagonal tasks; groups are dd-matched where possible, mismatched
    lanes zero-fill their sliver with a tiny warm matmul.
  - Masks: one [128,128] triangle tile, DVE tensor_mul (~150ns), off
    GpSimd/Scalar.
  - qt/kt drain fans out over Vector/Scalar/GpSimd; pair0's first two
    S groups are all-super-0 low-j so the corridor unblocks early.
  - Dense dep-free pads earn the HAM K=8/8 grant early and hold it
    through the store phase (postamble dispatches ~2x faster at K=8).
"""

import sys

if "/opt/trn_rl_repo" not in sys.path:
    sys.path.insert(0, "/opt/trn_rl_repo")

import os
from contextlib import ExitStack

import numpy as np

import concourse.bass as bass
import concourse.tile as tile
from concourse import bacc, mybir
from concourse.bass_utils import run_bass_kernel_spmd

B, T, D, HS = 8, 2048, 1024, 64
N_CORES = 8
F32 = mybir.dt.float32
BF16 = mybir.dt.bfloat16

TT = 128
NDT = D // TT
NTT = T // TT
QS = 512
PW = 2 * QS
VP = HS + 1
VPAD = 80

GROUPS0 = [
    ((0, 0), (1, 0)), ((2, 0), (3, 0)),
    ((0, 1), (1, 1)), ((2, 1), (3, 1)), ((4, 1), (5, 1)), ((6, 1), (7, 1)),
]
GROUPS1 = (
    [((j, 2), (j, 3)) for j in range(8)]
    + [((8 + i, 2), (12 + i, 3)) for i in range(4)]
    + [((8, 3), (9, 3)), ((10, 3), (11, 3))]
)
A_LAST = {0: 1, 1: 11}


def task_dd(j, s):
    dd = j - 4 * s
    return dd if 0 <= dd < 4 else -1


def group_dd(g):
    d0 = task_dd(*g[0])
    d1 = task_dd(*g[1])
    return min(d0 if d0 >= 0 else 0, d1 if d1 >= 0 else 0)


def build_graph() -> bacc.Bacc:
    nc = bacc.Bacc("TRN2", target_bir_lowering=False, debug=False)

    xt_ext = nc.dram_tensor("xt", [D, T], BF16, kind="ExternalInput").ap()
    wqk_ext = nc.dram_tensor("wqk", [TT, NDT * TT], BF16,
                             kind="ExternalInput").ap()
    wv_ext = nc.dram_tensor("wv", [TT, NDT * VP], BF16,
                            kind="ExternalInput").ap()
    bcol_ext = nc.dram_tensor("bcol", [TT, 2], F32, kind="ExternalInput").ap()
    out_ext = nc.dram_tensor("outT", [VP, T], F32, kind="ExternalOutput").ap()

    with tile.TileContext(nc) as tc, ExitStack() as ctx:
        const = ctx.enter_context(tc.tile_pool(name="const", bufs=1))
        persist = ctx.enter_context(tc.tile_pool(name="persist", bufs=1))
        xt_pool = ctx.enter_context(tc.tile_pool(name="xt", bufs=1))
        vt_pool = ctx.enter_context(tc.tile_pool(name="vt", bufs=2))
        pt_pool = ctx.enter_context(tc.tile_pool(name="pt", bufs=8))
        osb_pool = ctx.enter_context(tc.tile_pool(name="osb", bufs=2))
        warm_pool = ctx.enter_context(tc.tile_pool(name="warm", bufs=1))
        psum = ctx.enter_context(tc.tile_pool(name="ps", bufs=1, space="PSUM"))

        def proj_t(name):
            return psum.tile([TT, 2, QS], F32, tag="proj", bufs=2, name=name)

        def sp_t(name):
            return psum.tile([TT, 2, QS], F32, tag="sp", bufs=2, name=name)

        qt_sb = persist.tile([TT, T], BF16)
        kt_sb = persist.tile([TT, T], BF16)
        vp_sb = persist.tile([TT, NTT * VPAD], BF16)
        tri_sb = persist.tile([TT, TT], BF16)

        xt_sb = [
            xt_pool.tile([TT, PW], BF16, tag=f"xt{c}", bufs=1, name=f"xt{c}_0")
            for c in range(NDT)
        ]
        xt1_sb = xt_pool.tile([TT, NDT * PW], BF16, tag="xt1", name="xt1_all")
        wqk_sb = const.tile([TT, NDT * TT], BF16)
        wv_sb = const.tile([TT, NDT * VP], BF16)
        bcol_sb = const.tile([TT, 2], F32)

        # ring balance: scalar carries 5 x chunks (1.28MB), sync carries
        # wqk+x1+wv+x3+x5 (1.15MB) -- x7 on sync would land last (~21.8us)
        # and gate pqk0-stop
        for c in (0, 2, 4, 6, 7):
            nc.scalar.dma_start(
                xt_sb[c][:], xt_ext[c * TT:(c + 1) * TT, 0:PW]
            )
        nc.scalar.dma_start(bcol_sb[:], bcol_ext)

        nc.sync.dma_start(wqk_sb[:], wqk_ext)
        nc.sync.dma_start(xt_sb[1][:], xt_ext[TT:2 * TT, 0:PW])
        nc.sync.dma_start(wv_sb[:], wv_ext)
        for c in (3, 5):
            nc.sync.dma_start(
                xt_sb[c][:], xt_ext[c * TT:(c + 1) * TT, 0:PW]
            )
        for lo, hi in ((0, 2), (2, 5), (5, 8)):
            nc.sync.dma_start(
                xt1_sb[:, lo * PW:hi * PW].rearrange(
                    "p (c n) -> p c n", c=hi - lo
                ),
                xt_ext[lo * TT:hi * TT, PW:2 * PW].rearrange(
                    "(c p) n -> p c n", p=TT
                ),
            )

        warm_sb = warm_pool.tile([TT, QS], BF16)
        nc.gpsimd.memset(warm_sb[:], 0.0)
        nc.gpsimd.memset(tri_sb[:], 1.0)
        nc.gpsimd.affine_select(
            out=tri_sb[:], in_=tri_sb[:],
            compare_op=mybir.AluOpType.is_ge, fill=0.0,
            base=0, channel_multiplier=-1, pattern=[[1, TT]],
        )

        pad_state = {}

        def pad(n, new_tile=None):
            if new_tile is not None:
                pad_state["tile"] = new_tile
            pt_ = pad_state["tile"]
            for _ in range(n):
                nc.tensor.matmul(
                    pt_[:, 0, :], warm_sb[0:HS, 0:TT], warm_sb[0:HS, 0:QS],
                    start=True, stop=True, tile_position=(0, 0),
                    skip_group_check=True,
                )

        pad(8, new_tile=sp_t("pad0"))

        # ---- proj0: fused, V lagging QK by 2 chunks.  V(c-2) is READY
        # work emitted BEFORE the arrival-gated QK(c) so it fills the
        # x-chunk arrival gaps (program-order pads can't fill stalls of
        # instructions ahead of them) ----
        order = [0, 1, 2, 3, 4, 5, 6, 7]
        pqk0 = proj_t("pqk0")
        pv0 = proj_t("pv0")

        def proj_chunk(w_sb, pp, xts, c, first, last, wcols=TT):
            wsl = w_sb[:, c * wcols:(c + 1) * wcols]
            for xi in range(2):
                nc.tensor.matmul(
                    pp[0:wcols, xi, :], wsl,
                    xts[c][:, xi * QS:(xi + 1) * QS],
                    start=first, stop=last, skip_group_check=True,
                )

        for i, c in enumerate(order):
            if 2 <= i < NDT - 1:
                proj_chunk(wv_sb, pv0, xt_sb, order[i - 2], i == 2, False,
                           wcols=VP)
            proj_chunk(wqk_sb, pqk0, xt_sb, c, i == 0, i == NDT - 1)
            if i in (0, 1):
                pad(3)
        # QK(o7) goes AHEAD of the last three V chunks: pqk0 stops ~0.7us
        # earlier (the exp train start is gated on it) and the V tail
        # covers the drain corridor instead
        for j in (5, 6):
            proj_chunk(wv_sb, pv0, xt_sb, order[j], False, False, wcols=VP)
        proj_chunk(wv_sb, pv0, xt_sb, order[7], False, True, wcols=VP)

        def drain_qk(p, pqk):
            """qt/kt bias-add into BOTH partition halves.  Vector and
            Scalar each do two direct PSUM reads (GpSimd can't read PSUM
            and its SBUF copy costs 3.6us); the T0-lane feed (kt_lo,
            qt_lo) goes first on each engine.  For pair0 the Vector ops
            are split by column-half in consumer order: attn0's T0-only
            groups need kt cols [0:512] first; its dual groups only ever
            read qt_hi's super-b half."""
            cols = slice(p * PW, (p + 1) * PW)
            pq = pqk[:].rearrange("p g c -> p (g c)")
            if p == 0:
                # kt_lo halves (feed the T0-only groups) come first; the
                # caller interleaves vt0-add before qt_hi; kt_hi goes on
                # Scalar.  super-a's qt_hi half is never read.
                for lo, hi in ((0, QS), (QS, PW)):
                    nc.vector.tensor_scalar_add(
                        kt_sb[0:HS, lo:hi], pq[HS:2 * HS, lo:hi],
                        bcol_sb[HS:2 * HS, 0:1]
                    )
            else:
                nc.vector.tensor_scalar_add(
                    kt_sb[0:HS, cols], pq[HS:2 * HS, :],
                    bcol_sb[HS:2 * HS, 0:1]
                )
            nc.scalar.activation(
                qt_sb[0:HS, cols], pq[0:HS, :],
                mybir.ActivationFunctionType.Identity,
                bias=bcol_sb[0:HS, 0:1],
            )
            if p == 1:
                nc.vector.tensor_scalar_add(
                    qt_sb[HS:TT, cols], pq[0:HS, :], bcol_sb[0:HS, 0:1]
                )

            def emit_kt_hi():
                # kt_hi only feeds later dual-lane groups; deferring it
                # past exp(0) keeps the Identity out of the exp train's
                # critical start
                nc.scalar.activation(
                    kt_sb[HS:TT, cols], pq[HS:2 * HS, :],
                    mybir.ActivationFunctionType.Identity,
                    bias=bcol_sb[HS:2 * HS, 0:1],
                )

            return emit_kt_hi

        def drain_qt_hi0(pqk):
            pq = pqk[:].rearrange("p g c -> p (g c)")
            nc.vector.tensor_scalar_add(
                qt_sb[HS:TT, QS:PW], pq[0:HS, QS:PW], bcol_sb[0:HS, 0:1]
            )

        def vp_finish(p, pv):
            vt = vt_pool.tile([VPAD, PW], BF16, tag="vt", name=f"vt{p}")
            nc.gpsimd.memset(vt[HS:VPAD, :], 0.0)
            nc.vector.tensor_scalar_add(
                vt[0:VP, :], pv[:].rearrange("p g c -> p (g c)")[0:VP, :],
                bcol_sb[0:VP, 1:2]
            )
            nc.sync.dma_start_transpose(
                vp_sb[:, 8 * p * VPAD:(8 * p + 8) * VPAD].rearrange(
                    "q (c n) -> q c n", c=8
                ),
                vt[0:VPAD, :],
            )

        kt_hi0 = drain_qk(0, pqk0)
        vp_finish(0, pv0)
        drain_qt_hi0(pqk0)
        pad(4)

        # ---- attention: dual-lane S, single-lane PV, filler interleave
        def attn(p, groups, t0_only, ot, filler, after_exp0=None):
            a = 2 * p
            n = len(groups)
            spg, ptt = {}, {}
            total, seen = {}, {}
            for g in groups:
                for (j, s) in g:
                    total[s] = total.get(s, 0) + 1

            def emit_S(g):
                sp = sp_t(f"sp{p}_{g}")
                gdd = group_dd(groups[g])
                for lane, (j, s) in enumerate(groups[g]):
                    dd = max(task_dd(j, s), 0)
                    if g < t0_only or lane == 0:
                        tp, rows = (0, 0), slice(0, HS)
                    else:
                        tp, rows = (HS, 0), slice(HS, TT)
                    if dd > gdd:
                        nc.tensor.matmul(
                            sp[:, lane, gdd * TT:dd * TT],
                            warm_sb[rows.start:rows.start + HS, 0:TT],
                            warm_sb[rows.start:rows.start + HS,
                                    0:(dd - gdd) * TT],
                            start=True, stop=True, tile_position=tp,
                            skip_group_check=True,
                        )
                    nc.tensor.matmul(
                        sp[:, lane, dd * TT:QS],
                        kt_sb[rows, j * TT:(j + 1) * TT],
                        qt_sb[rows, s * QS + dd * TT:(s + 1) * QS],
                        start=True, stop=True, tile_position=tp,
                        skip_group_check=True,
                    )
                spg[g] = sp

            def emit_exp(g):
                gdd = group_dd(groups[g])
                pt = pt_pool.tile([TT, 2, QS], BF16, tag="pt",
                                  name=f"pt{p}_{g}")
                nc.scalar.activation(
                    pt[:, :, gdd * TT:QS],
                    spg.pop(g)[:, :, gdd * TT:QS],
                    mybir.ActivationFunctionType.Exp,
                )
                ptt[g] = pt

            def emit_masks(g):
                # GpSimd affine_select on the [128,128] diagonal block:
                # ~160ns on an otherwise-idle engine, off the Scalar and
                # Vector critical paths.
                pt = ptt[g]
                for lane, (j, s) in enumerate(groups[g]):
                    dd = task_dd(j, s)
                    if dd >= 0:
                        nc.gpsimd.affine_select(
                            out=pt[:, lane, dd * TT:(dd + 1) * TT],
                            in_=pt[:, lane, dd * TT:(dd + 1) * TT],
                            compare_op=mybir.AluOpType.is_ge,
                            fill=0.0, base=0, channel_multiplier=-1,
                            pattern=[[1, TT]],
                        )

            def emit_PV(g):
                pt = ptt.pop(g)
                for lane, (j, s) in enumerate(groups[g]):
                    dd = max(task_dd(j, s), 0)
                    half = s - a
                    k = seen[s] = seen.get(s, 0) + 1
                    nc.tensor.matmul(
                        ot[0:VP, half, dd * TT:QS],
                        vp_sb[:, j * VPAD:j * VPAD + VP],
                        pt[:, lane, dd * TT:QS],
                        start=(k == 1), stop=(k == total[s]),
                        skip_group_check=True,
                    )

            def store(s):
                half = s - a
                osb = osb_pool.tile([VP, QS], F32, tag="osb",
                                    name=f"osb{p}_{s}")
                nc.vector.tensor_copy(osb[:], ot[0:VP, half, :])
                eng = nc.scalar if s % 2 else nc.sync
                eng.dma_start(out_ext[:, s * QS:(s + 1) * QS], osb[:])

            emit_S(0)
            emit_S(1)
            emit_exp(0)
            if after_exp0 is not None:
                after_exp0()
            for g in range(n):
                emit_masks(g)
                if g + 2 < n:
                    emit_S(g + 2)
                if g + 1 < n:
                    emit_exp(g + 1)
                if filler is not None:
                    take = 3 if g < 4 else 2
                    for _ in range(take):
                        next(filler, None)
                emit_PV(g)
                if g == A_LAST[p]:
                    store(a)
            store(a + 1)

        def filler_proj(w_sb, pp, tail, wcols=TT):
            """Yield after each chunk's 2 matmuls; run tail() at the end."""
            for c in range(NDT):
                wsl = w_sb[:, c * wcols:(c + 1) * wcols]
                for xi in range(2):
                    nc.tensor.matmul(
                        pp[0:wcols, xi, :], wsl,
                        xt1_sb[:, c * PW + xi * QS:c * PW + (xi + 1) * QS],
                        start=(c == 0), stop=(c == NDT - 1),
                        skip_group_check=True,
                    )
                yield
            tail()
            yield

        ot0 = proj_t("ot0")
        pqk1 = proj_t("pqk1")
        kt_hi1 = {}

        def drain1():
            kt_hi1["emit"] = drain_qk(1, pqk1)

        attn(0, GROUPS0, t0_only=3, ot=ot0,
             filler=filler_proj(wqk_sb, pqk1, drain1),
             after_exp0=kt_hi0)

        pv1 = proj_t("pv1")
        ot1 = proj_t("ot1")
        attn(1, GROUPS1, t0_only=1, ot=ot1,
             filler=filler_proj(wv_sb, pv1, lambda: vp_finish(1, pv1),
                                wcols=VP),
             after_exp0=lambda: kt_hi1["emit"]())

        pad(12, new_tile=proj_t("padT"))

    nc.compile()
    return nc


def make_inputs(x_b, Wq, bq, Wk, bk, Wv, bv):
    """Host-side prep for one core's in_map (x_b: [T, D] fp32)."""
    import ml_dtypes

    bf = ml_dtypes.bfloat16
    scale = 1.0 / np.sqrt(np.float32(HS))
    wqk = np.zeros((D, TT), dtype=np.float32)
    wqk[:, 0:HS] = Wq * scale
    wqk[:, HS:2 * HS] = Wk
    wv = np.zeros((D, VP), dtype=np.float32)
    wv[:, 0:HS] = Wv

    def chunk_major(w):
        n = w.shape[1]
        return np.ascontiguousarray(
            w.reshape(NDT, TT, n).transpose(1, 0, 2).reshape(TT, NDT * n)
        )

    bcol = np.zeros((TT, 2), dtype=np.float32)
    bcol[0:HS, 0] = bq * scale
    bcol[HS:2 * HS, 0] = bk
    bcol[0:HS, 1] = bv
    bcol[HS, 1] = 1.0
    return {
        "xt": np.ascontiguousarray(x_b.T).astype(bf),
        "wqk": chunk_major(wqk).astype(bf),
        "wv": chunk_major(wv).astype(bf),
        "bcol": bcol,
    }


def finish_output(outT):
    """Host-side normalize + transpose: outT [65, T] -> [T, HS]."""
    o = np.asarray(outT, dtype=np.float32)
    return (o[0:HS, :] / o[HS:HS + 1, :]).T


_NC_CACHE = None


def _get_nc():
    global _NC_CACHE
    if _NC_CACHE is None:
        _NC_CACHE = build_graph()
    return _NC_CACHE


def kernel(x, Wq, bq, Wk, bk, Wv, bv):
    x = np.asarray(x, dtype=np.float32)
    args = [np.asarray(a, dtype=np.float32) for a in (Wq, bq, Wk, bk, Wv, bv)]
    nc = _get_nc()
    in_maps = [make_inputs(x[b], *args) for b in range(N_CORES)]
    trace = os.environ.get("BASS_ATTN_TRACE", "0") == "1"
    res = run_bass_kernel_spmd(
        nc, in_maps, core_ids=list(range(N_CORES)), trace=trace
    )
    if trace:
        print(
            f"HW exec time: {res.exec_time_ns} ns "
            f"(mean {res.mean_exec_time_ns}, max core {res.max_exec_time_core_id})"
        )
    out = np.stack(
        [finish_output(res.results[b]["outT"]) for b in range(N_CORES)], axis=0
    )
    return out
